# revision 2
# baseline (speedup 1.0000x reference)
"""Vocab-parallel cross-entropy loss kernel for Trainium2 (8 NeuronCores).

loss = sum_t w_t * (logsumexp_v(h_t . W_v) - h_t . W_{label_t}) / (sum_t w_t + 1e-8) / gacc

Sharding: head_weight split along vocab (32000 -> 8 x 4000). Every core computes
logits for all 4096 tokens against its vocab shard (bf16 matmul, fp32 PSUM
accumulation), reduces exp-sums and picked-logit partials on the fly, then one
32KB AllReduce combines the softmax normalizer and picked logits; each core
finishes the (identical) scalar loss and core 0's output is returned.

Self-contained: hardcodes shapes from the problem spec; only needs numpy,
ml_dtypes and the concourse (Bass/Tile) stack available in the container.
"""

import os

import numpy as np
import ml_dtypes

os.environ.setdefault("MYCRO_LOCAL_CACHE", "1")

import concourse.bass as bass  # noqa: E402
import concourse.tile as tile  # noqa: E402
from concourse import bacc  # noqa: E402
from concourse import mybir  # noqa: E402
from concourse.bass_utils import run_bass_kernel_spmd  # noqa: E402

F32 = mybir.dt.float32
BF16 = mybir.dt.bfloat16
ALU = mybir.AluOpType
ACTF = mybir.ActivationFunctionType
AX = mybir.AxisListType

# Problem shapes (hardcoded per contract).
B, S, H, V = 2, 2048, 4096, 32000
T = B * S                      # 4096 tokens
NCORES = 8
VL = V // NCORES               # 4000 vocab rows per core

P = 128                        # partitions
TT = T // P                    # 32 token tiles
HH = H // P                    # 32 contraction tiles
NSUP = 2                       # vocab supers per core (W^T SBUF residency)
VSUP = VL // NSUP              # 2000
CHUNK = 500                    # psum free dim (<=512 f32/bank)
NCH = VSUP // CHUNK            # 4 chunks per super
NCG = NSUP * NCH               # 8 chunks total per core

_CACHE = {}


def _build(n_passes=1, single_core=False, mm_order="ch", wt_bufs=34,
           hid_bufs=3, ep_bufs=3):
    nc = bacc.Bacc("TRN2", target_bir_lowering=False, debug=False,
                   num_devices=1 if single_core else NCORES)
    hidt = nc.dram_tensor("hidt", [H, T], BF16, kind="ExternalInput")
    wsh = nc.dram_tensor("wsh", [VL, H], BF16, kind="ExternalInput")
    ll = nc.dram_tensor("ll", [P, TT], F32, kind="ExternalInput")
    lw = nc.dram_tensor("lw", [P, TT], F32, kind="ExternalInput")
    loss = nc.dram_tensor("loss", [1, 1], F32, kind="ExternalOutput")

    hid_r = hidt.ap().rearrange("(ho p) t -> p ho t", p=P)

    with tile.TileContext(nc) as tc:
        wtp = tc.alloc_tile_pool(name="wtp", bufs=wt_bufs)
        hp = tc.alloc_tile_pool(name="hp", bufs=hid_bufs)
        ep = tc.alloc_tile_pool(name="ep", bufs=ep_bufs)
        pp = tc.alloc_tile_pool(name="pp", bufs=8, space="PSUM")
        cp = tc.alloc_tile_pool(name="cp", bufs=1)   # persistents/constants
        dp = tc.alloc_tile_pool(name="dp", bufs=1, space="DRAM")

        # ---- constants / persistents ----
        iota = cp.tile([P, CHUNK], F32, tag="iota")
        nc.gpsimd.iota(iota, pattern=[[1, CHUNK]], base=0, channel_multiplier=0,
                       allow_small_or_imprecise_dtypes=True)
        ll_sb = cp.tile([P, TT], F32, tag="ll")
        nc.sync.dma_start(out=ll_sb, in_=ll[:, :])
        lw_sb = cp.tile([P, TT], F32, tag="lw")
        nc.sync.dma_start(out=lw_sb, in_=lw[:, :])
        # shifted labels per chunk-group: col = cg*TT + t  -> ll - 500*cg
        ll_shift = cp.tile([P, NCG * TT], F32, tag="llsh")
        for cg in range(NCG):
            nc.vector.tensor_scalar_add(ll_shift[:, cg * TT:(cg + 1) * TT],
                                        ll_sb, float(-CHUNK * cg))
        # per-(t, chunk-group) partial sums, overwritten once each
        zacc = cp.tile([P, NCG * TT], F32, tag="zacc")
        pacc = cp.tile([P, NCG * TT], F32, tag="pacc")

        # ---- main loop ----
        for s in [s for _ in range(n_passes) for s in range(NSUP)]:
            v0 = s * VSUP
            wts = []
            for h in range(HH):
                wt_t = wtp.tile([P, VSUP], BF16, tag="wt")
                nc.sync.dma_start(out=wt_t,
                                  in_=wsh[v0:v0 + VSUP, h * P:(h + 1) * P],
                                  transpose=True)
                wts.append(wt_t)
            for t in range(TT):
                hid_sb = hp.tile([P, HH, P], BF16, tag="hid")
                nc.sync.dma_start(out=hid_sb, in_=hid_r[:, :, t * P:(t + 1) * P])
                ps = [pp.tile([P, CHUNK], F32, tag="ps", name=f"ps{c}")
                      for c in range(NCH)]
                if mm_order == "hc":
                    for h in range(HH):
                        lhsT = hid_sb[:, h, :]
                        for c in range(NCH):
                            nc.tensor.matmul(ps[c], lhsT=lhsT,
                                             rhs=wts[h][:, c * CHUNK:(c + 1) * CHUNK],
                                             start=(h == 0), stop=(h == HH - 1))
                else:  # "ch": psum bank fixed across the h accumulation chain
                    for c in range(NCH):
                        for h in range(HH):
                            nc.tensor.matmul(ps[c], lhsT=hid_sb[:, h, :],
                                             rhs=wts[h][:, c * CHUNK:(c + 1) * CHUNK],
                                             start=(h == 0), stop=(h == HH - 1))
                for c in range(NCH):
                    cg = s * NCH + c
                    col = cg * TT + t
                    esc = ep.tile([P, CHUNK], F32, tag="esc")
                    nc.scalar.activation(esc, ps[c], func=ACTF.Exp,
                                         accum_out=zacc[:, col:col + 1])
                    psc = ep.tile([P, CHUNK], F32, tag="psc")
                    nc.vector.scalar_tensor_tensor(
                        out=psc, in0=iota, scalar=ll_shift[:, col:col + 1],
                        in1=ps[c], op0=ALU.is_equal, op1=ALU.mult,
                        accum_out=pacc[:, col:col + 1])

        # ---- reduce partials and all-reduce ----
        arin = cp.tile([P, 2 * TT], F32, tag="arin")
        nc.vector.reduce_sum(out=arin[:, 0:TT],
                             in_=zacc[:].rearrange("p (c t) -> p t c", c=NCG),
                             axis=AX.X)
        nc.vector.reduce_sum(out=arin[:, TT:2 * TT],
                             in_=pacc[:].rearrange("p (c t) -> p t c", c=NCG),
                             axis=AX.X)
        arsum = cp.tile([P, 2 * TT], F32, tag="arsum")
        if single_core:
            nc.vector.tensor_copy(arsum[:], arin[:])
        else:
            ar_in = dp.tile([P, 2 * TT], F32, tag="ari")
            ar_out = dp.tile([P, 2 * TT], F32, tag="aro")
            nc.gpsimd.dma_start(out=ar_in[:], in_=arin[:, :])
            nc.gpsimd.collective_compute(
                "AllReduce", ALU.add, replica_groups=[list(range(NCORES))],
                ins=[ar_in.opt()], outs=[ar_out.opt()])
            nc.gpsimd.dma_start(out=arsum[:], in_=ar_out[:])

        # ---- finale: loss = sum(w*(log Z - picked)) / (sum w + 1e-8) ----
        logz = cp.tile([P, TT], F32, tag="logz")
        nc.scalar.activation(logz, arsum[:, 0:TT], func=ACTF.Ln)
        pt = cp.tile([P, TT], F32, tag="pt")
        nc.vector.tensor_tensor(pt, logz, arsum[:, TT:2 * TT], ALU.subtract)
        ptw = cp.tile([P, TT], F32, tag="ptw")
        nc.vector.tensor_tensor(ptw, pt, lw_sb, ALU.mult)
        stats2 = cp.tile([P, 2], F32, tag="stats2")
        nc.vector.reduce_sum(out=stats2[:, 0:1], in_=ptw, axis=AX.X)
        nc.vector.reduce_sum(out=stats2[:, 1:2], in_=lw_sb, axis=AX.X)
        ones = cp.tile([P, 1], F32, tag="ones")
        nc.vector.memset(ones, 1.0)
        ps2 = pp.tile([P, CHUNK], F32, tag="ps")
        nc.tensor.matmul(ps2[:1, :2], lhsT=ones[:, 0:1], rhs=stats2[:, 0:2],
                         start=True, stop=True)
        res = cp.tile([1, 4], F32, tag="res")
        nc.vector.tensor_scalar_add(res[:, 1:2], ps2[:1, 1:2], 1e-8)
        nc.vector.reciprocal(res[:, 2:3], res[:, 1:2])
        nc.vector.tensor_tensor(res[:, 0:1], ps2[:1, 0:1], res[:, 2:3], ALU.mult)
        nc.sync.dma_start(out=loss[:, :], in_=res[:, 0:1])

        dp.release(); cp.release(); pp.release(); ep.release()
        hp.release(); wtp.release()

    nc.compile()
    return nc


def _get_nc():
    if "nc" not in _CACHE:
        _CACHE["nc"] = _build()
    return _CACHE["nc"]


def kernel(hidden_states, head_weight, labels, loss_weight,
           grad_accumulation_steps):
    hid = np.asarray(hidden_states, dtype=np.float32).reshape(T, H)
    W = np.asarray(head_weight, dtype=np.float32)
    lab = np.asarray(labels).reshape(-1).astype(np.int64)
    lwf = np.asarray(loss_weight, dtype=np.float32).reshape(-1)
    g = np.asarray(grad_accumulation_steps, dtype=np.float64).reshape(-1)
    gacc = float(g[0]) if g.size else 1.0

    # host layout prep: bf16 cast; hidden transposed to [H, T]
    hidt = np.ascontiguousarray(hid.T).astype(ml_dtypes.bfloat16)
    lw2 = np.ascontiguousarray(lwf.reshape(TT, P).T)  # [p, t_tile]

    in_maps = []
    for c in range(NCORES):
        llc = lab - c * VL
        llc = np.where((llc >= 0) & (llc < VL), llc, -1).astype(np.float32)
        in_maps.append({
            "hidt": hidt,
            "wsh": np.ascontiguousarray(W[c * VL:(c + 1) * VL]).astype(ml_dtypes.bfloat16),
            "ll": np.ascontiguousarray(llc.reshape(TT, P).T),
            "lw": lw2,
        })

    nc = _get_nc()
    _CACHE["in_maps"] = in_maps
    res = run_bass_kernel_spmd(nc, in_maps, core_ids=list(range(NCORES)),
                               trace=False)
    _CACHE["last_results"] = res
    out = np.float32(res.results[0]["loss"][0, 0] / gacc)
    return np.asarray(out, dtype=np.float32)



# revision 3
# speedup vs baseline: 1239.0923x; 1239.0923x over previous
"""Vocab-parallel cross-entropy loss kernel for Trainium2 (8 NeuronCores).

fp8 DoubleRow edition, wide drains: logits are computed in fp8e4 (scaled) with
perf_mode=DoubleRow matmuls (K=256 per matmul: lhsT [128,2,128] hidden tokens
stationary, rhs [128,2,500] weight chunk moving), halving PE streaming cycles
vs bf16.  PSUM is a single [128,8,512] f32 region (8 bank-aligned chunks);
after each 4-chunk group's accumulation chains finish, ONE wide activation
(exp + accumulate) and ONE wide scalar_tensor_tensor (picked-logit select)
drain the whole group via strided [128,4,500-of-512] APs, overlapping the
other group's matmul chains.

Scaling: hidden x ASCALE, weights x WSCALE on host; exp() applies 1/(A*W)
via the activation scale; the picked-logit partial is descaled after the
32KB AllReduce.  All host-side layout prep (transpose/pack/cast) is free
w.r.t. HW exec time.

Self-contained: hardcodes shapes from the problem spec.
"""

import os

import numpy as np
import ml_dtypes

os.environ.setdefault("MYCRO_LOCAL_CACHE", "1")

import concourse.bass as bass  # noqa: E402
import concourse.tile as tile  # noqa: E402
from concourse import bacc  # noqa: E402
from concourse import mybir  # noqa: E402
from concourse.bass_utils import run_bass_kernel_spmd  # noqa: E402

F32 = mybir.dt.float32
BF16 = mybir.dt.bfloat16
FP8 = mybir.dt.float8e4
ALU = mybir.AluOpType
ACTF = mybir.ActivationFunctionType
AX = mybir.AxisListType
PERF = mybir.MatmulPerfMode

# Problem shapes (hardcoded per contract).
B, S, H, V = 2, 2048, 4096, 32000
T = B * S                      # 4096 tokens
NCORES = 8
VL = V // NCORES               # 4000 vocab rows per core

P = 128                        # partitions
TT = T // P                    # 32 token tiles
KK = H // 256                  # 16 k-tiles of 256 (DoubleRow)
CHUNK = 500                    # psum free dim (<=512 f32/bank)
NCH = VL // CHUNK              # 8 chunks
CPAD = 512                     # padded chunk pitch inside the wt tile
G = 2                          # chunk groups per token tile (drain staggering)
GC = NCH // G                  # 4 chunks per group

ASCALE = 4.0                   # hidden fp8 scale
WSCALE = 256.0                 # weight fp8 scale
SINV = 1.0 / (ASCALE * WSCALE)

HROW = KK * 2 * P              # 4096 fp8 bytes per partition per token tile
WROW = 2 * NCH * CPAD          # 8192 fp8 bytes per partition per k tile

_CACHE = {}


def _build(n_passes=1, single_core=False, wt_bufs=18, hid_bufs=3, ep_bufs=3,
           skel_passes=0):
    """skel_passes: extra timing-calibration passes with the identical
    instruction mix but near-zero HW work (tiny moving dims / 16B DMAs).
    Used to cancel any per-instruction dispatch overhead when measuring
    the main-loop time as (x4 - x1_plus_3skel)/3.  Output is garbage when
    skel_passes > 0 - timing use only."""
    nc = bacc.Bacc("TRN2", target_bir_lowering=False, debug=False,
                   num_devices=1 if single_core else NCORES)
    hidp = nc.dram_tensor("hidp", [P, TT * HROW], FP8, kind="ExternalInput")
    wshp = nc.dram_tensor("wshp", [P, KK * WROW], FP8, kind="ExternalInput")
    ll = nc.dram_tensor("ll", [P, TT], F32, kind="ExternalInput")
    lw = nc.dram_tensor("lw", [P, TT], F32, kind="ExternalInput")
    loss = nc.dram_tensor("loss", [1, 1], F32, kind="ExternalOutput")

    with tile.TileContext(nc) as tc:
        wtp = tc.alloc_tile_pool(name="wtp", bufs=wt_bufs)
        hp = tc.alloc_tile_pool(name="hp", bufs=hid_bufs)
        ep = tc.alloc_tile_pool(name="ep", bufs=ep_bufs)
        ep2 = tc.alloc_tile_pool(name="ep2", bufs=ep_bufs)
        pp = tc.alloc_tile_pool(name="pp", bufs=2, space="PSUM")
        cp = tc.alloc_tile_pool(name="cp", bufs=1)   # persistents/constants
        dp = tc.alloc_tile_pool(name="dp", bufs=1, space="DRAM")

        # ---- constants / persistents ----
        GW = GC * CHUNK            # 2000 logits per drain group
        iota = cp.tile([P, GW], F32, tag="iota")
        nc.gpsimd.iota(iota, pattern=[[1, GW]], base=0, channel_multiplier=0,
                       allow_small_or_imprecise_dtypes=True)
        ll_sb = cp.tile([P, TT], F32, tag="ll")
        nc.sync.dma_start(out=ll_sb, in_=ll[:, :])
        lw_sb = cp.tile([P, TT], F32, tag="lw")
        nc.sync.dma_start(out=lw_sb, in_=lw[:, :])
        # shifted labels per drain group: col = g*TT + t  -> ll - 2000*g
        ll_shift = cp.tile([P, G * TT], F32, tag="llsh")
        for g in range(G):
            nc.vector.tensor_scalar_add(ll_shift[:, g * TT:(g + 1) * TT],
                                        ll_sb, float(-GW * g))
        # per-(t, group) partial sums, overwritten once each
        zacc = cp.tile([P, G * TT], F32, tag="zacc")
        pacc = cp.tile([P, G * TT], F32, tag="pacc")

        # ---- main loop ----
        for ip in range(n_passes + skel_passes):
            skel = ip >= n_passes
            wts = []
            for k in range(KK):
                wt_t = wtp.tile([P, 2, NCH, CPAD], FP8, tag="wt")
                if skel:
                    nc.sync.dma_start(out=wt_t[:, 0, 0, 0:16],
                                      in_=wshp.ap()[:, 0:16])
                else:
                    nc.sync.dma_start(
                        out=wt_t,
                        in_=wshp.ap()[:, k * WROW:(k + 1) * WROW].rearrange(
                            "p (i c v) -> p i c v", i=2, c=NCH))
                wts.append(wt_t)
            for t in range(TT):
                hid_sb = hp.tile([P, KK, 2, P], FP8, tag="hid")
                if skel:
                    nc.sync.dma_start(out=hid_sb[:, 0, 0, 0:16],
                                      in_=hidp.ap()[:, 0:16])
                else:
                    nc.sync.dma_start(
                        out=hid_sb,
                        in_=hidp.ap()[:, t * HROW:(t + 1) * HROW].rearrange(
                            "p (k i q) -> p k i q", k=KK, i=2))
                # one PSUM tile per drain group: 4 bank-aligned 512-f32 chunks
                ps = [pp.tile([P, GC, 512], F32, tag="ps", name=f"ps{g}")
                      for g in range(G)]
                for g in range(G):
                    for k in range(KK):
                        lhsT = hid_sb[:, k]          # [P, 2, 128]
                        for ci in range(GC):
                            if skel:
                                nc.tensor.matmul(
                                    ps[g][0:2, ci, 0:2], lhsT=lhsT[:, :, 0:2],
                                    rhs=wts[k][:, :, g * GC + ci, 0:2],
                                    start=(k == 0), stop=(k == KK - 1),
                                    perf_mode=PERF.DoubleRow)
                            else:
                                nc.tensor.matmul(
                                    ps[g][:, ci, 0:CHUNK], lhsT=lhsT,
                                    rhs=wts[k][:, :, g * GC + ci, 0:CHUNK],
                                    start=(k == 0), stop=(k == KK - 1),
                                    perf_mode=PERF.DoubleRow)
                    col = g * TT + t
                    w = 2 if skel else CHUNK
                    drain = ps[g][:, 0:1, 0:w] if skel else ps[g][:, :, 0:w]
                    iview = (iota[:, 0:w].rearrange("p (c v) -> p c v", c=1)
                             if skel else
                             iota[:, :].rearrange("p (c v) -> p c v", c=GC))
                    esc = ep.tile([P, GC, CHUNK], BF16, tag="esc")
                    nc.scalar.activation(esc[:, 0:1, 0:w] if skel else esc,
                                         drain, func=ACTF.Exp, scale=SINV,
                                         accum_out=zacc[:, col:col + 1])
                    psc = ep2.tile([P, GC, CHUNK], BF16, tag="psc")
                    nc.vector.scalar_tensor_tensor(
                        out=psc[:, 0:1, 0:w] if skel else psc,
                        in0=iview, scalar=ll_shift[:, col:col + 1],
                        in1=drain, op0=ALU.is_equal, op1=ALU.mult,
                        accum_out=pacc[:, col:col + 1])

        # ---- reduce partials and all-reduce ----
        arin = cp.tile([P, 2 * TT], F32, tag="arin")
        nc.vector.reduce_sum(out=arin[:, 0:TT],
                             in_=zacc[:].rearrange("p (c t) -> p t c", c=G),
                             axis=AX.X)
        nc.vector.reduce_sum(out=arin[:, TT:2 * TT],
                             in_=pacc[:].rearrange("p (c t) -> p t c", c=G),
                             axis=AX.X)
        arsum = cp.tile([P, 2 * TT], F32, tag="arsum")
        if single_core:
            nc.vector.tensor_copy(arsum[:], arin[:])
        else:
            ar_in = dp.tile([P, 2 * TT], F32, tag="ari")
            ar_out = dp.tile([P, 2 * TT], F32, tag="aro")
            nc.gpsimd.dma_start(out=ar_in[:], in_=arin[:, :])
            nc.gpsimd.collective_compute(
                "AllReduce", ALU.add, replica_groups=[list(range(NCORES))],
                ins=[ar_in.opt()], outs=[ar_out.opt()])
            nc.gpsimd.dma_start(out=arsum[:], in_=ar_out[:])

        # ---- finale: loss = sum(w*(log Z - picked)) / (sum w + 1e-8) ----
        logz = cp.tile([P, TT], F32, tag="logz")
        nc.scalar.activation(logz, arsum[:, 0:TT], func=ACTF.Ln)
        pk = cp.tile([P, TT], F32, tag="pk")
        nc.vector.tensor_scalar_mul(pk, arsum[:, TT:2 * TT], SINV)
        pt = cp.tile([P, TT], F32, tag="pt")
        nc.vector.tensor_tensor(pt, logz, pk, ALU.subtract)
        ptw = cp.tile([P, TT], F32, tag="ptw")
        nc.vector.tensor_tensor(ptw, pt, lw_sb, ALU.mult)
        stats2 = cp.tile([P, 2], F32, tag="stats2")
        nc.vector.reduce_sum(out=stats2[:, 0:1], in_=ptw, axis=AX.X)
        nc.vector.reduce_sum(out=stats2[:, 1:2], in_=lw_sb, axis=AX.X)
        ones = cp.tile([P, 1], F32, tag="ones")
        nc.vector.memset(ones, 1.0)
        ps2 = pp.tile([P, GC, 512], F32, tag="ps")
        nc.tensor.matmul(ps2[:1, 0, :2], lhsT=ones[:, 0:1], rhs=stats2[:, 0:2],
                         start=True, stop=True)
        res = cp.tile([1, 4], F32, tag="res")
        nc.vector.tensor_scalar_add(res[:, 1:2], ps2[:1, 0, 1:2], 1e-8)
        nc.vector.reciprocal(res[:, 2:3], res[:, 1:2])
        nc.vector.tensor_tensor(res[:, 0:1], ps2[:1, 0, 0:1], res[:, 2:3], ALU.mult)
        nc.sync.dma_start(out=loss[:, :], in_=res[:, 0:1])

        dp.release(); cp.release(); pp.release(); ep2.release(); ep.release()
        hp.release(); wtp.release()

    nc.compile()
    return nc


def _get_nc():
    if "nc" not in _CACHE:
        _CACHE["nc"] = _build()
    return _CACHE["nc"]


def _f8(x):
    return np.clip(x, -240.0, 240.0).astype(ml_dtypes.float8_e4m3)


def kernel(hidden_states, head_weight, labels, loss_weight,
           grad_accumulation_steps):
    hid = np.asarray(hidden_states, dtype=np.float32).reshape(T, H)
    W = np.asarray(head_weight, dtype=np.float32)
    lab = np.asarray(labels).reshape(-1).astype(np.int64)
    lwf = np.asarray(loss_weight, dtype=np.float32).reshape(-1)
    g = np.asarray(grad_accumulation_steps, dtype=np.float64).reshape(-1)
    gacc = float(g[0]) if g.size else 1.0

    # hidden: scale+cast, pack to hidp[p, t, k, i, tok] (per-partition rows
    # contiguous per token tile)
    h8 = _f8(hid * ASCALE)                       # [T, H]
    hidp = np.ascontiguousarray(
        h8.reshape(TT, P, KK, 2, P).transpose(4, 0, 2, 3, 1)
    ).reshape(P, TT * HROW)
    lw2 = np.ascontiguousarray(lwf.reshape(TT, P).T)  # [p, t_tile]

    in_maps = []
    for c in range(NCORES):
        w8 = _f8(W[c * VL:(c + 1) * VL] * WSCALE)     # [VL, H]
        warr = np.zeros((P, KK, 2, NCH, CPAD), ml_dtypes.float8_e4m3)
        warr[:, :, :, :, 0:CHUNK] = w8.reshape(
            NCH, CHUNK, KK, 2, P).transpose(4, 2, 3, 0, 1)
        llc = lab - c * VL
        llc = np.where((llc >= 0) & (llc < VL), llc, -1).astype(np.float32)
        in_maps.append({
            "hidp": hidp,
            "wshp": np.ascontiguousarray(warr).reshape(P, KK * WROW),
            "ll": np.ascontiguousarray(llc.reshape(TT, P).T),
            "lw": lw2,
        })

    nc = _get_nc()
    _CACHE["in_maps"] = in_maps
    res = run_bass_kernel_spmd(nc, in_maps, core_ids=list(range(NCORES)),
                               trace=False)
    _CACHE["last_results"] = res
    out = np.float32(res.results[0]["loss"][0, 0] / gacc)
    return np.asarray(out, dtype=np.float32)
